# revision 16
# baseline (speedup 1.0000x reference)
"""CRF negative-log-likelihood loss on 8 Trainium2 NeuronCores.

Strategy (time-parallel chunked scan):
  - The T=2048 forward recursion is split into 8 chunks of 256 steps, one per
    core, each preceded by a 33-step warmup: the CRF forward map is a strict
    Birkhoff contraction (~0.4x/step here), so the normalized state forgets
    its initialization to < 1e-12 within 33 steps. Core 0's warmup columns are
    fabricated identity-ish steps (transition basis collapses to the all-ones
    matrix), which makes its trajectory exact from t=0.
  - Per-step transition kernel exp(trans[i,j] * s) (s = 1/weight in
    [smin,smax]) is approximated by a rank-4 basis: B_0 = ones plus the top-3
    SVD factors of the family {exp(trans*s) - 1 : s in range}; measured
    end-to-end relative error ~2e-8 (dominated by nothing else).
  - Exp-domain state A (unnormalized forward probabilities) with a constant
    2^-6 per-step rescale plus an exact reciprocal rescale every 32 steps;
    the per-step normalizer logs telescope into per-chunk scalars combined
    on the host.
  - Per step and 128-batch block: one DVE tensor_tensor builds
    V2[w,(k,i)] = A[w,i] * g_k(s), one PE transpose flips it to [(k,i),w],
    one ACT copy moves it PSUM->SBUF, one PE matmul contracts (k,i) against
    the constant basis stack, and one DVE scalar_tensor_tensor applies
    rescale and the exp(emission) column factor.
  - Gold-path emission score (incl. start_transitions at t=0) is computed on
    device via one-hot compare + multiply-accumulate against the raw
    emission tiles; the tiny O(T*B) transition/end gather runs on host.
"""

import numpy as np

T, B, M = 2048, 256, 32
NCORE = 8
WIN = T // NCORE          # 256
WARM = 33
L = WIN + WARM + 1        # 290 columns (col 0 = init column)
K = 4
CONST_RS = 2.0 ** -6
RS = 32                   # true-rescale period (columns j % 32 == 0)
RANGES = [(0, 64), (64, 128), (128, 192), (192, 256), (256, L)]
WSTART = WARM + 1         # first window column (34)
NZ = L // RS              # 9 true-rescale columns: j = 32, 64, ..., 288
# zpack columns: [0:NZ] Z at j=32..288, [NZ] SA_pre (j=33), [NZ+1] SA_end
# (j=289), [NZ+2 : NZ+2+5] em-score partial sums per range
NC_SCORE = len(RANGES)
NCOLS = NZ + 2 + NC_SCORE

_prog_cache = {}


def _build_program():
    import concourse.bass as bass
    import concourse.bacc as bacc
    import concourse.tile as tile
    from concourse import mybir
    from concourse.masks import make_identity

    f32 = mybir.dt.float32
    nc = bacc.Bacc()

    em_d = nc.dram_tensor("em", [B, L, M], f32, kind="ExternalInput")
    g_d = nc.dram_tensor("gc", [B, L, K], f32, kind="ExternalInput")
    tg_d = nc.dram_tensor("tg", [B, WIN], f32, kind="ExternalInput")
    ch_d = nc.dram_tensor("chat", [K * M, M], f32, kind="ExternalInput")
    io_d = nc.dram_tensor("iota32", [128, M], f32, kind="ExternalInput")
    zp_d = nc.dram_tensor("zpack", [B, NCOLS], f32, kind="ExternalOutput")
    ae_d = nc.dram_tensor("aend", [B, M], f32, kind="ExternalOutput")

    with tile.TileContext(nc) as tc:
        import contextlib
        ctx = contextlib.ExitStack()
        with ctx:
            singles = ctx.enter_context(tc.tile_pool(name="singles", bufs=1))
            em_pool = ctx.enter_context(tc.tile_pool(name="em", bufs=2))
            emx0_pool = ctx.enter_context(tc.tile_pool(name="emx0", bufs=2))
            g_pool = ctx.enter_context(tc.tile_pool(name="g", bufs=2))
            tg_pool = ctx.enter_context(tc.tile_pool(name="tg", bufs=2))
            zc_pool = ctx.enter_context(tc.tile_pool(name="zc", bufs=2))
            v1_pool = ctx.enter_context(tc.tile_pool(name="v1", bufs=4))
            v2_pool = ctx.enter_context(tc.tile_pool(name="v2", bufs=4))
            v128_pool = ctx.enter_context(tc.tile_pool(name="v128", bufs=4))
            rc_pool = ctx.enter_context(tc.tile_pool(name="rc", bufs=4))
            oh_pool = ctx.enter_context(tc.tile_pool(name="oh", bufs=2))
            ps_s = ctx.enter_context(tc.tile_pool(name="ps_s", bufs=3, space="PSUM"))
            ps_t = ctx.enter_context(tc.tile_pool(name="ps_t", bufs=3, space="PSUM"))

            ident = singles.tile([128, 128], f32)
            make_identity(nc, ident)
            chat_t = singles.tile([128, M], f32)
            nc.sync.dma_start(out=chat_t, in_=ch_d[:, :])
            iota_t = singles.tile([128, M], f32)
            nc.sync.dma_start(out=iota_t, in_=io_d[:, :])

            em_t = {}       # (blk, ri) -> raw-em tile (score reads these)
            emx_t = {}      # (blk, ri) -> exp(em) tile (scan reads these)
            g_t, tg_t, zc_t = {}, {}, {}
            dscr_pool = ctx.enter_context(tc.tile_pool(name="dscr", bufs=24))
            for blk in range(2):
                b0 = blk * 128
                for ri, (r0, r1) in enumerate(RANGES):
                    t_ = em_pool.tile([128, r1 - r0, M], f32, tag=f"em{ri}", name=f"em{ri}")
                    nc.sync.dma_start(out=t_, in_=em_d[b0:b0 + 128, r0:r1, :])
                    em_t[(blk, ri)] = t_
                g_t[blk] = g_pool.tile([128, L, K], f32, tag="gt", name="gt")
                nc.sync.dma_start(out=g_t[blk], in_=g_d[b0:b0 + 128, :, :])
                tg_t[blk] = tg_pool.tile([128, WIN], f32, tag="tgt", name="tgt")
                nc.sync.dma_start(out=tg_t[blk], in_=tg_d[b0:b0 + 128, :])
                zc_t[blk] = zc_pool.tile([128, NCOLS], f32, tag="zct", name="zct")
                # exp'd copies of every range: exp waits only on its DMA
                for ri, (r0, r1) in enumerate(RANGES):
                    x_ = emx0_pool.tile([128, r1 - r0, M], f32, tag=f"emx{ri}", name=f"emx{ri}")
                    nc.scalar.activation(
                        out=x_.rearrange("p a b -> p (a b)"),
                        in_=em_t[(blk, ri)].rearrange("p a b -> p (a b)"),
                        func=mybir.ActivationFunctionType.Exp,
                    )
                    emx_t[(blk, ri)] = x_

            # Pre-consume DMA semaphores on DVE with tiny 2D copies so that the
            # 3D-AP DVE ops below never need more than one sync-wait (the
            # S3S3D3 encodings have a single wait slot).
            def dve_touch(src_ap):
                d = dscr_pool.tile([128, 1], f32, tag="dscr", name="dscr")
                nc.vector.tensor_copy(out=d, in_=src_ap)

            def act_touch(src_ap):
                d = dscr_pool.tile([128, 1], f32, tag="ascr", name="ascr")
                nc.scalar.copy(out=d, in_=src_ap)

            dve_touch(iota_t[:, 0:1])
            for blk in range(2):
                dve_touch(g_t[blk][:, 0, 0:1])
                dve_touch(tg_t[blk][:, 0:1])
                for ri in range(len(RANGES)):
                    dve_touch(em_t[(blk, ri)][:, 0, 0:1])
                    dve_touch(emx_t[(blk, ri)][:, 0, 0:1])

            # PE pre-consumers: absorb identity + basis-stack semaphores.
            pe_t0 = ps_t.tile([128, 128], f32, tag="pet0", name="pet0", bufs=1)
            nc.tensor.transpose(out=pe_t0, in_=ident, identity=ident)
            pe_s0 = ps_s.tile([128, M], f32, tag="pes0", name="pes0", bufs=1)
            nc.tensor.matmul(pe_s0, ident, chat_t, start=True, stop=True)

            # ---- em-score (reads RAW em) + in-place exp for ranges >= 1 ----
            # Emitted before the scan so the in-place exp (a write over raw
            # em) is ordered after the raw reads; the scheduler still
            # overlaps everything that is ready.
            for blk in range(2):
                for ri, (r0, r1) in enumerate(RANGES):
                    w0 = max(r0, WSTART)
                    n = r1 - w0
                    oh = oh_pool.tile([128, RANGES[0][1], M], f32, tag="oh", name="oh")
                    ohv = oh[:, :n, :]
                    tg_ap = (
                        tg_t[blk][:, w0 - WSTART:w0 - WSTART + n]
                        .unsqueeze(2)
                        .broadcast_to([128, n, M])
                    )
                    io_ap = iota_t.unsqueeze(1).broadcast_to([128, n, M])
                    nc.vector.tensor_tensor(
                        out=ohv, in0=tg_ap, in1=io_ap,
                        op=mybir.AluOpType.is_equal,
                    )
                    nc.vector.scalar_tensor_tensor(
                        out=ohv,
                        in0=em_t[(blk, ri)][:, w0 - r0:, :],
                        scalar=1.0,
                        in1=ohv,
                        op0=mybir.AluOpType.mult,
                        op1=mybir.AluOpType.mult,
                        accum_out=zc_t[blk][:, NZ + 2 + ri:NZ + 3 + ri],
                    )


            # ---- the serial scan, both 128-batch blocks interleaved ----
            prev = [emx_t[(0, 0)][:, 0, :], emx_t[(1, 0)][:, 0, :]]
            for j in range(1, L):
                ri = min(j // 64, len(RANGES) - 1)
                r0 = RANGES[ri][0]
                for blk in range(2):
                    emx_ap = emx_t[(blk, ri)][:, j - r0, :]

                    v2 = v2_pool.tile([128, K, M], f32, tag="v2", name="v2")
                    nc.vector.tensor_tensor(
                        out=v2,
                        in0=prev[blk].unsqueeze(1).broadcast_to([128, K, M]),
                        in1=g_t[blk][:, j, :].unsqueeze(2).broadcast_to([128, K, M]),
                        op=mybir.AluOpType.mult,
                    )
                    v2t = ps_t.tile([128, 128], f32, tag="v2t", name="v2t")
                    nc.tensor.transpose(
                        out=v2t,
                        in_=v2.rearrange("p k i -> p (k i)"),
                        identity=ident,
                    )
                    v128 = v128_pool.tile([128, 128], f32, tag="v128", name="v128")
                    nc.scalar.copy(out=v128, in_=v2t)
                    if j % 4 == 0:
                        # advance ACT's observed self-tick so later v128-slot
                        # WAW deps are already satisfied (1-wait limit).
                        act_touch(v128[:, 0:1])
                    s_ps = ps_s.tile([128, M], f32, tag="sps", name="sps")
                    nc.tensor.matmul(s_ps, v128, chat_t, start=True, stop=True)

                    if j % RS == 0:
                        zcol = zc_t[blk][:, j // RS - 1:j // RS]
                        nc.vector.reduce_sum(
                            out=zcol, in_=s_ps, axis=mybir.AxisListType.X
                        )
                        rc = rc_pool.tile([128, 1], f32, tag="rc", name="rc")
                        nc.vector.reciprocal(out=rc, in_=zcol)
                        scal = rc[:, :]
                    else:
                        scal = CONST_RS

                    accum = None
                    if j == WARM:
                        accum = zc_t[blk][:, NZ:NZ + 1]
                    elif j == L - 1:
                        accum = zc_t[blk][:, NZ + 1:NZ + 2]
                    v1 = v1_pool.tile([128, M], f32, tag="v1", name="v1")
                    nc.vector.scalar_tensor_tensor(
                        out=v1,
                        in0=s_ps,
                        scalar=scal,
                        in1=emx_ap,
                        op0=mybir.AluOpType.mult,
                        op1=mybir.AluOpType.mult,
                        accum_out=accum,
                    )
                    prev[blk] = v1[:, :]

            for blk in range(2):
                b0 = blk * 128
                nc.sync.dma_start(out=ae_d[b0:b0 + 128, :], in_=prev[blk])
                nc.sync.dma_start(out=zp_d[b0:b0 + 128, :], in_=zc_t[blk])

    nc.finalize()
    return nc


def _host_prep(em, s, trans, st):
    """Build per-core input packs. Returns (in_maps, n_const_logs)."""
    smin, smax = float(s.min()), float(s.max())
    if smax - smin < 1e-9:
        smax = smin + 1e-6
    sg = np.linspace(smin, smax, 64)
    G = np.exp(trans.astype(np.float64).reshape(-1)[None, :] * sg[:, None]) - 1.0
    U, S, Vt = np.linalg.svd(G, full_matrices=False)
    r = K - 1
    US = U[:, :r] * S[None, :r]
    Bas = np.concatenate([np.ones((1, M * M)), Vt[:r]], 0).reshape(K, M, M)
    polys = [np.polynomial.polynomial.Polynomial.fit(sg, US[:, k], 7)
             for k in range(r)]

    chat = Bas.reshape(K * M, M).astype(np.float32)  # [(k,i), j], k-major
    iota = np.tile(np.arange(M, dtype=np.float32).reshape(1, M), (128, 1))

    # g columns for every t: g[t] = g(s[t-1]) used by arrival at time t
    g_all = np.empty((T, B, K), np.float32)
    g_all[1:, :, 0] = 1.0
    sv = s[: T - 1].astype(np.float64)
    for k in range(r):
        g_all[1:, :, k + 1] = polys[k](sv).astype(np.float32)
    g_all[0] = 0.0
    g_all[0, :, 0] = 1.0  # t=0 arrival: identity-ish fake (ones basis only)

    em0 = (em[0] + st[None, :]).astype(np.float32)

    in_maps = []
    for c in range(NCORE):
        em_pack = np.empty((B, L, M), np.float32)
        g_pack = np.empty((B, L, K), np.float32)
        t_lo = c * WIN - (WARM + 1)
        for j in range(L):
            t = t_lo + j
            if t <= 0:
                em_pack[:, j, :] = em0
                g_pack[:, j, :] = 0.0
                g_pack[:, j, 0] = 1.0
            else:
                em_pack[:, j, :] = em[t]
                g_pack[:, j, :] = g_all[t]
        tg_pack = np.ascontiguousarray(
            tags_f32_global[c * WIN:(c + 1) * WIN].T
        )
        in_maps.append({
            "em": em_pack, "gc": g_pack, "tg": tg_pack,
            "chat": chat, "iota32": iota,
        })
    return in_maps


tags_f32_global = None


def _numpy_fallback(emissions, tags, weight, mask, transitions,
                    start_transitions, end_transitions):
    em = emissions.astype(np.float64)
    tg = tags.astype(np.int64)
    w = weight.astype(np.float64)
    mk = mask.astype(bool)
    tr = transitions.astype(np.float64)
    st = start_transitions.astype(np.float64)
    et = end_transitions.astype(np.float64)
    Tn, Bn, Mn = em.shape
    tg = np.where(mk, tg, 1)
    mf = mk.astype(np.float64)

    score = st[tg[0]]
    score = score + (tr[tg[:-1], tg[1:]] * mf[1:] / w[:-1]).sum(0)
    score = score + (np.take_along_axis(em, tg[:, :, None], -1)[..., 0] * mf).sum(0)
    seq_ends = mk.astype(np.int64).sum(0) - 1
    score = score + et[tg[seq_ends, np.arange(Bn)]]

    def lse(x, axis):
        m = x.max(axis=axis, keepdims=True)
        return (m + np.log(np.exp(x - m).sum(axis=axis, keepdims=True))).squeeze(axis)

    alpha = st[None, :] + em[0]
    for t in range(1, Tn):
        sc = tr[None, :, :] / w[t - 1][:, None, None] + em[t][:, None, :]
        new = lse(alpha[:, :, None] + sc, 1)
        alpha = np.where(mk[t][:, None], new, alpha)
    logZ = lse(alpha + et[None, :], 1)
    return np.float32((logZ - score).sum())


def kernel(**inputs):
    global tags_f32_global
    em = np.ascontiguousarray(np.asarray(inputs["emissions"], np.float32))
    tags = np.asarray(inputs["tags"]).astype(np.int64)
    weight = np.asarray(inputs["weight"], np.float32)
    mask = np.asarray(inputs["mask"])
    trans = np.asarray(inputs["transitions"], np.float32)
    st = np.asarray(inputs["start_transitions"], np.float32)
    et = np.asarray(inputs["end_transitions"], np.float32)

    if not bool((np.asarray(mask) == 1).all()):
        return _numpy_fallback(em, tags, weight, mask, trans, st, et)

    s = (1.0 / weight.astype(np.float64)).astype(np.float32)
    tags_f32_global = tags.astype(np.float32)

    in_maps = _host_prep(em, s, trans, st)

    if "prog" not in _prog_cache:
        _prog_cache["prog"] = _build_program()
    nc = _prog_cache["prog"]

    from concourse.bass_utils import run_bass_kernel_spmd
    res = run_bass_kernel_spmd(nc, in_maps, core_ids=list(range(NCORE)))
    outs = res.results

    zp = np.stack([outs[c]["zpack"] for c in range(NCORE)], 0).astype(np.float64)
    ae = outs[NCORE - 1]["aend"].astype(np.float64)

    n_const = WIN - (NZ - 1)  # window arrivals minus true-rescales in window
    logacc = np.log(zp[:, :, 1:NZ]).sum(-1) + n_const * (-np.log(CONST_RS))
    D = np.log(zp[:, :, NZ + 1]) - np.log(zp[:, :, NZ]) + logacc
    logZ = D.sum(0)
    logZ = logZ + np.log((ae * np.exp(et.astype(np.float64))[None, :]).sum(1))
    logZ = logZ - np.log(ae.sum(1))

    em_score = zp[:, :, NZ + 2:].sum((0, 2))  # [B]
    s64 = 1.0 / weight.astype(np.float64)
    tr_score = (trans.astype(np.float64)[tags[:-1], tags[1:]]
                * s64[:-1]).sum(0)
    score = em_score + tr_score + et.astype(np.float64)[tags[-1]]

    return np.float32((logZ - score).sum())
